# revision 1
# baseline (speedup 1.0000x reference)
"""NT-Xent loss kernel for Trainium2 (8 NeuronCores, SPMD row-sharded).

Math:
    zi, zj  = L2-normalized rows of z_i, z_j          (host, trivial)
    z       = concat(zi, zj)            [2B, D]       (host)
    rs[r]   = sum_c exp(2 * <z_r, z_c>)               (device)
    loss    = -mean( 2*<zi_k, zj_k> - log(rs[k] + rs[k+B] - 2*e^2) )

Because rows are unit-norm, the diagonal of the similarity matrix is exactly
exp(2): no masking on device, the host subtracts it.

Device algorithm ("sym2"): symmetric/triangle coverage with a two-engine exp
pipeline.  The 16384x16384 exp-similarity matrix is symmetric; global
128-row m-tiles are assigned to cores in mirror pairs (M, 127-M) for exact
per-core balance with an SPMD-uniform instruction schedule.  An m-tile in
diagonal super-block j computes exp only for columns [DIAG*j, 16384).

Each schedule chunk is built as TWO independent PSUM tiles (separate banks):
  - head [0:1024): PE fp16 matmuls fill it; the Scalar (ACT) engine applies
    Exp (PSUM -> fp16 E in SBUF) with the fused row-sum accumulator.
  - tail: PE fills it; the Vector (DVE) engine applies a Schraudolph
    bit-trick exp: ONE fused tensor_scalar (S*c1 + c2 -> int16, round) whose
    output bits ARE the fp16 encoding of ~exp(S/T) (max ~3% per-element
    error, mean-centered via SCH_DELTA; row sums of 16K entries average it
    out).  GpSimd halves the codes (fp16 adds), DVE reduces the final
    quarter, deferred two chunks so DVE never stalls on GpSimd.
Separate head/tail tiles keep ACT's and DVE's PSUM write-after-read loops
independent - this decoupling is what lets all engines run near-saturated.

Column sums for strictly-upper 128-subtiles come from tiny PE matmuls
(lhsT = E fp16 or bitcast codes, rhs = ones) accumulated into a PSUM
scratch bank, drained per m-tile into an SBUF accumulator by DVE.  Row-sum
partial columns (ACT accums + DVE tail reduces) are DMA'd out raw; the host
does the final O(n_chunks) sums in float64.
"""
import sys

import numpy as np

sys.path.insert(0, "/opt/trn_rl_repo")

TEMPERATURE = 0.5
B = 8192
D = 128
N_CORES = 8
NFULL = 2 * B           # 16384 rows of z
RPC = NFULL // N_CORES  # 2048 rows per core (flash impl)
P = 128                 # partitions
CHUNK = 2048            # flash impl ACT chunk width

DIAG = 1024   # diagonal super-block width; (DIAG//P) % N_CORES == 0 required
WA = 2048     # chunk A: 1024 ACT head (2 banks) + 1024 DVE tail (2 banks)
WB = 1536     # chunk B: 1024 ACT head (2 banks) + 512 DVE tail (1 bank)

_PROGRAM_CACHE = {}

# Filled in by the most recent kernel() call when _trace=True.
LAST_EXEC_NS = None
LAST_RESULTS = None


def build_program(nfull=NFULL, rpc=RPC, chunk=CHUNK):
    """Simple full-matrix (flash) variant; kept as a fallback."""
    import concourse.bacc as bacc
    import concourse.tile as tile
    from concourse import mybir

    f32 = mybir.dt.float32
    f16 = mybir.dt.float16
    nc = bacc.Bacc("TRN2", target_bir_lowering=False)
    zT = nc.dram_tensor("zT", [P, nfull], f16, kind="ExternalInput")
    blkT = nc.dram_tensor("blkT", [P, rpc], f16, kind="ExternalInput")
    mt = rpc // P
    nch = nfull // chunk
    rs_dram = nc.dram_tensor("rs", [P, mt], f32, kind="ExternalOutput")

    with tile.TileContext(nc) as tc:
        with (
            tc.tile_pool(name="zfull", bufs=1) as zfull_pool,
            tc.tile_pool(name="blk", bufs=1) as blk_pool,
            tc.tile_pool(name="psum", bufs=2, space="PSUM") as psum_pool,
            tc.tile_pool(name="rsch", bufs=3) as rs_pool,
            tc.tile_pool(name="outp", bufs=1) as out_pool,
        ):
            ZW = min(2048, nfull)
            zparts = []
            for i in range(nfull // ZW):
                zp = zfull_pool.tile([P, ZW], f16, tag=f"z{i}")
                nc.sync.dma_start(out=zp[:], in_=zT[:, i * ZW : (i + 1) * ZW])
                zparts.append(zp)

            def z_slice(c0, w):
                i, off = c0 // ZW, c0 % ZW
                assert off + w <= ZW
                return zparts[i][:, off : off + w]

            blk_sb = blk_pool.tile([P, rpc], f16)
            nc.sync.dma_start(out=blk_sb[:], in_=blkT[:, :])

            rs_all = out_pool.tile([P, mt], f32)
            for m in range(mt):
                rs_ch = rs_pool.tile([P, nch], f32)
                for g in range(nch):
                    ps = psum_pool.tile([P, chunk], f32)
                    for k in range(chunk // 512):
                        nc.tensor.matmul(
                            out=ps[:, k * 512 : (k + 1) * 512],
                            lhsT=blk_sb[:, m * P : (m + 1) * P],
                            rhs=z_slice(g * chunk + k * 512, 512),
                            start=True,
                            stop=True,
                        )
                    nc.scalar.activation(
                        out=ps[:],
                        in_=ps[:],
                        func=mybir.ActivationFunctionType.Exp,
                        scale=1.0 / TEMPERATURE,
                        accum_out=rs_ch[:, g : g + 1],
                    )
                nc.vector.tensor_reduce(
                    out=rs_all[:, m : m + 1],
                    in_=rs_ch[:],
                    axis=mybir.AxisListType.X,
                    op=mybir.AluOpType.add,
                )
            nc.sync.dma_start(out=rs_dram[:, :], in_=rs_all[:])
    nc.compile()
    return nc


def _mtiles_for_core(c, n_mt, ncores):
    """Mirror-paired assignment: m-tiles M and n_mt-1-M share a core."""
    half = n_mt // (2 * ncores)
    first = [ncores * t + c for t in range(half)]
    return first + [n_mt - 1 - m for m in first]


def _sym_schedule(nfull, ncores, diag, wa=WA, wb=WB):
    """Per-core chunk schedule, identical on every core (asserted)."""
    n_mt = nfull // P
    mt = n_mt // ncores
    js = []
    for lm in range(mt):
        vals = {
            (P * _mtiles_for_core(c, n_mt, ncores)[lm]) // diag
            for c in range(ncores)
        }
        assert len(vals) == 1, f"schedule not SPMD-uniform at lm={lm}: {vals}"
        js.append(vals.pop())
    sched = []
    toggle = 0
    for lm in range(mt):
        c0 = diag * js[lm]
        chunks = []
        while c0 < nfull:
            w = min(wa if toggle == 0 else wb, nfull - c0)
            chunks.append((c0, w, toggle))
            toggle ^= 1
            c0 += w
        sched.append(chunks)
    return js, sched


def build_program_sym(nfull=NFULL, ncores=N_CORES, diag=DIAG, wa=WA, wb=WB,
                      colsum=True):
    import concourse.bacc as bacc
    import concourse.tile as tile
    from concourse import mybir

    f32 = mybir.dt.float32
    f16 = mybir.dt.float16
    n_mt = nfull // P
    mt = n_mt // ncores
    js, sched = _sym_schedule(nfull, ncores, diag, wa, wb)
    max_chunks = max(len(s) for s in sched)
    max_elig = (nfull - diag) // P  # widest per-m-tile colsum scratch

    nc = bacc.Bacc("TRN2", target_bir_lowering=False)
    zT = nc.dram_tensor("zT", [P, nfull], f16, kind="ExternalInput")
    blkT = nc.dram_tensor("blkT", [P, mt * P], f16, kind="ExternalInput")
    rs_dram = nc.dram_tensor("rs", [P, mt], f32, kind="ExternalOutput")
    cs_dram = nc.dram_tensor("cs", [P, n_mt], f32, kind="ExternalOutput")

    with tile.TileContext(nc) as tc:
        with (
            tc.tile_pool(name="zfull", bufs=1) as zfull_pool,
            tc.tile_pool(name="blk", bufs=1) as blk_pool,
            tc.tile_pool(name="pa", bufs=1, space="PSUM") as pa_pool,
            tc.tile_pool(name="pb", bufs=1, space="PSUM") as pb_pool,
            tc.tile_pool(name="csp", bufs=1, space="PSUM") as cs_pool,
            tc.tile_pool(name="epool", bufs=6) as e_pool,
            tc.tile_pool(name="rsch", bufs=6) as rs_pool,
            tc.tile_pool(name="outp", bufs=1) as out_pool,
            tc.tile_pool(name="singles", bufs=1) as singles,
        ):
            # blk first: the very first matmuls need only its first 128 cols,
            # so give m-tile 0's slice its own tile (own DMA dependency)
            blk0_sb = blk_pool.tile([P, P], f16, tag="blk0")
            nc.sync.dma_start(out=blk0_sb[:], in_=blkT[:, 0:P])
            blk_sb = blk_pool.tile([P, mt * P], f16)
            nc.sync.dma_start(out=blk_sb[:, P:], in_=blkT[:, P:])

            def blk_slice(lm):
                if lm == 0:
                    return blk0_sb[:]
                return blk_sb[:, lm * P : (lm + 1) * P]

            ZW = min(1024, nfull)
            zparts = []
            for i in range(nfull // ZW):
                zp = zfull_pool.tile([P, ZW], f16, tag=f"z{i}")
                nc.sync.dma_start(out=zp[:], in_=zT[:, i * ZW : (i + 1) * ZW])
                zparts.append(zp)

            def z_slice(c0, w):
                i, off = c0 // ZW, c0 % ZW
                assert off + w <= ZW
                return zparts[i][:, off : off + w]

            ones = singles.tile([P, 1], f16)
            nc.vector.memset(ones, 1.0)

            cs_acc = singles.tile([P, n_mt], f32, tag="cs_acc")
            nc.vector.memset(cs_acc[:], 0.0)


            rs_all = out_pool.tile([P, mt], f32)

            # Software-pipelined emission: colsum matmuls for chunk i are
            # emitted between mains of chunk i+1 and its ACT, so PE never
            # waits on ACT inside the steady-state loop (the colsums' E
            # dependency is a full chunk old by the time PE reaches them).
            scratch_by_lm = {}
            pendings = []  # [(lm, e, [(off, sidx)], n_total, is_last)]
            PEND_DEPTH = 3

            def flush_one():
                plm, pe_tile, offs, n_total, is_last = pendings.pop(0)
                scratch = scratch_by_lm[plm]
                for off, sidx in offs:
                    nc.tensor.matmul(
                        out=scratch[:, sidx : sidx + 1],
                        lhsT=pe_tile[:, off : off + P],
                        rhs=ones[:],
                        start=(sidx == 0),
                        stop=(sidx == n_total - 1),
                    )
                if is_last:
                    cstart_p = diag * (js[plm] + 1)
                    t0 = cstart_p // P
                    nc.vector.tensor_add(
                        cs_acc[:, t0 : t0 + n_total],
                        cs_acc[:, t0 : t0 + n_total],
                        scratch[:, 0:n_total],
                    )

            def flush_pending(all=False):
                while pendings and (all or len(pendings) >= PEND_DEPTH):
                    flush_one()

            for lm in range(mt):
                rs_ch = rs_pool.tile([P, max_chunks], f32)
                nchunks = len(sched[lm])
                cstart = diag * (js[lm] + 1)
                n_elig_total = max(0, (nfull - cstart) // P)
                i_elig = 0
                for ci, (c0, w, tg) in enumerate(sched[lm]):
                    pool, tag = (pa_pool, "pa") if tg == 0 else (pb_pool, "pb")
                    ps = pool.tile([P, w], f32, tag=tag)
                    for k in range(0, w, 512):
                        nc.tensor.matmul(
                            out=ps[:, k : k + 512],
                            lhsT=blk_slice(lm),
                            rhs=z_slice(c0 + k, 512),
                            start=True,
                            stop=True,
                        )
                    flush_pending()
                    e = e_pool.tile([P, w], f16, tag="e")
                    nc.scalar.activation(
                        out=e[:],
                        in_=ps[:],
                        func=mybir.ActivationFunctionType.Exp,
                        scale=1.0 / TEMPERATURE,
                        accum_out=rs_ch[:, ci : ci + 1],
                    )
                    offs = []
                    for off in range(0, w, P):
                        if c0 + off < cstart:
                            continue
                        offs.append((off, i_elig))
                        i_elig += 1
                    if colsum and offs:
                        if lm not in scratch_by_lm:
                            scratch_by_lm[lm] = cs_pool.tile(
                                [P, max_elig], f32, tag="css", name=f"css{lm}"
                            )
                        pendings.append(
                            (lm, e, offs, n_elig_total, i_elig == n_elig_total)
                        )
                    if ci == nchunks - 1:
                        nc.vector.tensor_reduce(
                            out=rs_all[:, lm : lm + 1],
                            in_=rs_ch[:, :nchunks],
                            axis=mybir.AxisListType.X,
                            op=mybir.AluOpType.add,
                        )
            flush_pending(all=True)
            nc.sync.dma_start(out=rs_dram[:, :], in_=rs_all[:])
            nc.sync.dma_start(out=cs_dram[:, :], in_=cs_acc[:])
    nc.compile()
    return nc


LOG2E = float(np.log2(np.e))
SCH_DELTA = -49.0          # Schraudolph fp16 bias tweak (centers mean rel err)


def _head_w(w):
    return min(w, 1024)


# rs columns per chunk: ACT accum (1) + DVE tail TTR (1 if tail)
def _chunk_cols(w):
    return 1 + (1 if w > 1024 else 0)


def build_program_sym2(nfull=NFULL, ncores=N_CORES, diag=DIAG, wa=WA, wb=WB):
    """Head/tail split-chunk exp with independent PSUM tiles per consumer.

    Each schedule chunk [c0, c0+w) is computed as two PSUM tiles:
      - head [0:1024): its own 2-bank tile, consumed by ACT (Exp -> fp16 E,
        fused row-sum accum into rsa).
      - tail [1024:w): its own tile (2 banks for wa-chunks, 1 bank for
        wb-chunks), consumed by DVE: one fused tensor_scalar applies the
        Schraudolph bit-trick (int16 codes whose bits are fp16 ~exp(S/T)),
        then one tensor_tensor_reduce adds the two code halves and reduces
        them into the tail row-sum (rsd).  DVE's chain is self-contained:
        no cross-engine waits on its queue.
    Separate head/tail tiles mean ACT's PSUM WAR loop never waits on DVE
    and vice versa.  Col sums for strictly-upper 128-subtiles via tiny PE
    matmuls (lhsT = E fp16 or bitcast codes) into a PSUM scratch bank,
    drained per m-tile into an SBUF accumulator by DVE.  Row-sum partial
    columns are DMA'd out raw per m-tile; the host does the final sums.
    """
    import concourse.bacc as bacc
    import concourse.tile as tile
    from concourse import mybir

    f32 = mybir.dt.float32
    f16 = mybir.dt.float16
    i16 = mybir.dt.int16
    n_mt = nfull // P
    mt = n_mt // ncores
    js, sched = _sym_schedule(nfull, ncores, diag, wa, wb)
    max_elig = (nfull - diag) // P
    max_na = max(len(s) for s in sched)
    max_nd = max(sum(1 for (_, w, _) in s if w > 1024) for s in sched)
    max_ncols = max_na + max_nd

    nc = bacc.Bacc("TRN2", target_bir_lowering=False)
    zT = nc.dram_tensor("zT", [P, nfull], f16, kind="ExternalInput")
    blkT = nc.dram_tensor("blkT", [P, mt * P], f16, kind="ExternalInput")
    rs_dram = nc.dram_tensor("rs", [P, mt * max_ncols], f32, kind="ExternalOutput")
    cs_dram = nc.dram_tensor("cs", [P, n_mt], f32, kind="ExternalOutput")

    with tile.TileContext(nc) as tc:
        with (
            tc.tile_pool(name="zfull", bufs=1) as zfull_pool,
            tc.tile_pool(name="blk", bufs=1) as blk_pool,
            tc.tile_pool(name="pah", bufs=1, space="PSUM") as pah_pool,
            tc.tile_pool(name="pat", bufs=1, space="PSUM") as pat_pool,
            tc.tile_pool(name="pbh", bufs=1, space="PSUM") as pbh_pool,
            tc.tile_pool(name="pbt", bufs=1, space="PSUM") as pbt_pool,
            tc.tile_pool(name="csp", bufs=1, space="PSUM") as cs_pool,
            tc.tile_pool(name="epool", bufs=6) as e_pool,
            tc.tile_pool(name="eipool", bufs=6) as ei_pool,
            tc.tile_pool(name="hpool", bufs=8) as h_pool,
            tc.tile_pool(name="rsch", bufs=4) as rs_pool,
            tc.tile_pool(name="singles", bufs=1) as singles,
        ):
            blk0_sb = blk_pool.tile([P, P], f16, tag="blk0")
            nc.sync.dma_start(out=blk0_sb[:], in_=blkT[:, 0:P])
            blk_sb = blk_pool.tile([P, mt * P], f16)
            nc.sync.dma_start(out=blk_sb[:, P:], in_=blkT[:, P:])

            def blk_slice(lm):
                if lm == 0:
                    return blk0_sb[:]
                return blk_sb[:, lm * P : (lm + 1) * P]

            ZW = 1024
            z_sb = zfull_pool.tile([P, nfull], f16)
            for i in range(nfull // ZW):
                # separate DMAs so early chunks' deps resolve quickly
                nc.sync.dma_start(
                    out=z_sb[:, i * ZW : (i + 1) * ZW],
                    in_=zT[:, i * ZW : (i + 1) * ZW],
                )

            def z_slice(c0, w):
                return z_sb[:, c0 : c0 + w]

            ones = singles.tile([P, 1], f16)
            nc.vector.memset(ones, 1.0)

            cs_acc = singles.tile([P, n_mt], f32, tag="cs_acc")
            nc.vector.memset(cs_acc[:], 0.0)


            scratch_by_lm = {}
            pendings = []  # [(lm, eview, [(off, sidx)], n_total, last)]
            PEND_DEPTH = 3

            def flush_one():
                plm, eview, offs, n_total, is_last = pendings.pop(0)
                scratch = scratch_by_lm[plm]
                for off, sidx in offs:
                    nc.tensor.matmul(
                        out=scratch[:, sidx : sidx + 1],
                        lhsT=eview(off),
                        rhs=ones[:],
                        start=(sidx == 0),
                        stop=(sidx == n_total - 1),
                    )
                if is_last:
                    cstart_p = diag * (js[plm] + 1)
                    t0 = cstart_p // P
                    nc.vector.tensor_add(
                        cs_acc[:, t0 : t0 + n_total],
                        cs_acc[:, t0 : t0 + n_total],
                        scratch[:, 0:n_total],
                    )

            def flush_pending(all=False):
                while pendings and (all or len(pendings) >= PEND_DEPTH):
                    flush_one()

            SCALE = (1.0 / TEMPERATURE) * 1024.0 * LOG2E
            BIAS = 15360.0 + SCH_DELTA
            red_q = []  # deferred DVE tail reduces

            for lm in range(mt):
                rsa = rs_pool.tile([P, max_na], f32, tag="rsa")
                rsd = rs_pool.tile([P, max_nd], f32, tag="rsd")
                nchunks = len(sched[lm])
                cstart = diag * (js[lm] + 1)
                n_elig_total = max(0, (nfull - cstart) // P)
                i_elig = 0
                ca = 0
                cd = 0
                for ci, (c0, w, tg) in enumerate(sched[lm]):
                    hw_ = _head_w(w)
                    tw = w - hw_
                    hpool = pah_pool if tg == 0 else pbh_pool
                    tpool = pat_pool if tg == 0 else pbt_pool
                    psh = hpool.tile([P, hw_], f32, tag="h")
                    for k in range(0, hw_, 512):
                        nc.tensor.matmul(
                            out=psh[:, k : k + 512],
                            lhsT=blk_slice(lm),
                            rhs=z_slice(c0 + k, 512),
                            start=True,
                            stop=True,
                        )
                    if tw > 0:
                        pst = tpool.tile([P, tw], f32, tag="t")
                        for k in range(0, tw, 512):
                            nc.tensor.matmul(
                                out=pst[:, k : k + 512],
                                lhsT=blk_slice(lm),
                                rhs=z_slice(c0 + hw_ + k, 512),
                                start=True,
                                stop=True,
                            )
                    flush_pending()
                    e = e_pool.tile([P, hw_], f16, tag="e")
                    nc.scalar.activation(
                        out=e[:],
                        in_=psh[:],
                        func=mybir.ActivationFunctionType.Exp,
                        scale=1.0 / TEMPERATURE,
                        accum_out=rsa[:, ca : ca + 1],
                    )
                    ca += 1
                    if tw > 0:
                        ei = ei_pool.tile([P, tw], i16, tag="ei")
                        nc.vector.tensor_scalar(
                            out=ei[:],
                            in0=pst[:],
                            scalar1=SCALE,
                            scalar2=BIAS,
                            op0=mybir.AluOpType.mult,
                            op1=mybir.AluOpType.add,
                        )
                        red = ei[:].bitcast(f16)
                        if tw >= 512:
                            m2 = tw // 2
                            h = h_pool.tile([P, m2], f16, tag="h")
                            nc.gpsimd.tensor_add(
                                h[:], red[:, 0:m2], red[:, m2:tw]
                            )
                            red = h[:]
                            if m2 >= 512:
                                m4 = m2 // 2
                                h2 = h_pool.tile([P, m4], f16, tag="h2")
                                nc.gpsimd.tensor_add(
                                    h2[:], red[:, 0:m4], red[:, m4:m2]
                                )
                                red = h2[:]
                        # defer the reduce two chunks so DVE never waits on
                        # GpSimd's halvings inline
                        red_q.append((red, rsd, cd))
                        cd += 1
                        if len(red_q) > 2:
                            r_ap, r_rs, r_col = red_q.pop(0)
                            nc.vector.tensor_reduce(
                                out=r_rs[:, r_col : r_col + 1],
                                in_=r_ap,
                                axis=mybir.AxisListType.X,
                                op=mybir.AluOpType.add,
                            )
                    e_ap = e[:]
                    ei_ap = ei[:] if tw > 0 else None

                    def eview(off, _e=e_ap, _ei=ei_ap, _x=hw_):
                        if off < _x:
                            return _e[:, off : off + P]
                        return _ei[:, off - _x : off - _x + P].bitcast(f16)

                    offs = []
                    for off in range(0, w, P):
                        if c0 + off < cstart:
                            continue
                        offs.append((off, i_elig))
                        i_elig += 1
                    if offs:
                        if lm not in scratch_by_lm:
                            scratch_by_lm[lm] = cs_pool.tile(
                                [P, max_elig], f32, tag="css", name=f"css{lm}"
                            )
                        pendings.append(
                            (lm, eview, offs, n_elig_total, i_elig == n_elig_total)
                        )
                    if ci == nchunks - 1:
                        while red_q:
                            r_ap, r_rs, r_col = red_q.pop(0)
                            nc.vector.tensor_reduce(
                                out=r_rs[:, r_col : r_col + 1],
                                in_=r_ap,
                                axis=mybir.AxisListType.X,
                                op=mybir.AluOpType.add,
                            )
                        base = lm * max_ncols
                        nc.sync.dma_start(
                            out=rs_dram[:, base : base + ca],
                            in_=rsa[:, 0:ca],
                        )
                        if cd:
                            nc.sync.dma_start(
                                out=rs_dram[:, base + ca : base + ca + cd],
                                in_=rsd[:, 0:cd],
                            )
            flush_pending(all=True)
            nc.sync.dma_start(out=cs_dram[:, :], in_=cs_acc[:])
    nc.compile()
    return nc, max_ncols, max_elig


def _pack_dr(z8T_cols):
    """[128, n] fp8-bytes (uint8) -> [64, 2, n] DoubleRow layout."""
    n = z8T_cols.shape[1]
    return np.ascontiguousarray(
        z8T_cols.reshape(2, 64, n).transpose(1, 0, 2)
    )


def _normalize(x):
    x = np.asarray(x, dtype=np.float32)
    n = np.sqrt((x * x).sum(axis=1, keepdims=True))
    return x / np.maximum(n, np.float32(1e-12))


def _finish_loss(rs, zi, zj):
    """rs: [2B] row sums including the diagonal term."""
    diag = np.exp(np.float64(1.0 / TEMPERATURE))
    rs64 = rs.astype(np.float64) - diag
    denom = rs64[:B] + rs64[B:]
    pos_logit = (zi.astype(np.float64) * zj.astype(np.float64)).sum(axis=1) * (
        1.0 / TEMPERATURE
    )
    loss = -(pos_logit - np.log(denom)).mean()
    return np.float32(loss)


def _run_with_retry(nc, in_maps, core_ids, trace):
    """One retry on transient device errors (rare NRT_EXEC_UNIT blips)."""
    from concourse.bass_utils import run_bass_kernel_spmd

    last_err = None
    for attempt in range(3):
        try:
            return run_bass_kernel_spmd(nc, in_maps, core_ids, trace=trace)
        except Exception as e:  # noqa: BLE001
            last_err = e
            if attempt == 2:
                raise
            import time

            time.sleep(2.0)
    raise last_err


def kernel(z_i, z_j, _trace=False, impl="sym2"):
    global LAST_EXEC_NS, LAST_RESULTS

    zi = _normalize(z_i)
    zj = _normalize(z_j)
    z = np.concatenate([zi, zj], axis=0)      # [2B, D] fp32
    zT = np.ascontiguousarray(z.T.astype(np.float16))  # [D=128, 2B]

    if impl == "sym2":
        key = ("sym2", NFULL, N_CORES, DIAG, WA, WB)
        if key not in _PROGRAM_CACHE:
            _PROGRAM_CACHE[key] = build_program_sym2(NFULL, N_CORES, DIAG, WA, WB)
        nc, max_ncols, max_elig = _PROGRAM_CACHE[key]

        n_mt = NFULL // P
        mt = n_mt // N_CORES
        js, sched = _sym_schedule(NFULL, N_CORES, DIAG, WA, WB)
        in_maps = []
        core_mtiles = []
        for c in range(N_CORES):
            mtiles = _mtiles_for_core(c, n_mt, N_CORES)
            core_mtiles.append(mtiles)
            blk = np.concatenate(
                [zT[:, M * P : (M + 1) * P] for M in mtiles], axis=1
            )
            in_maps.append({"zT": zT, "blkT": np.ascontiguousarray(blk)})

        res = _run_with_retry(nc, in_maps, list(range(N_CORES)), _trace)
        LAST_EXEC_NS = res.exec_time_ns
        LAST_RESULTS = res

        rs_full = np.zeros(NFULL, dtype=np.float64)
        cs_tot = np.zeros((P, n_mt), dtype=np.float64)
        for c in range(N_CORES):
            rs_raw = res.results[c]["rs"].astype(np.float64)  # [P, mt*max_ncols]
            cs_tot += res.results[c]["cs"]
            for lm, M in enumerate(core_mtiles[c]):
                ncols = sum(_chunk_cols(w) for (_, w, _) in sched[lm])
                rs_full[M * P : (M + 1) * P] += rs_raw[
                    :, lm * max_ncols : lm * max_ncols + ncols
                ].sum(axis=1)
        cs_tot[:, : DIAG // P] = 0.0
        rs_full += cs_tot.T.reshape(-1)
        return _finish_loss(rs_full, zi, zj)

    if impl == "flash":
        key = (NFULL, RPC, CHUNK)
        if key not in _PROGRAM_CACHE:
            _PROGRAM_CACHE[key] = build_program(*key)
        nc = _PROGRAM_CACHE[key]
        in_maps = []
        for c in range(N_CORES):
            blk = np.ascontiguousarray(zT[:, c * RPC : (c + 1) * RPC])
            in_maps.append({"zT": zT, "blkT": blk})
        res = _run_with_retry(nc, in_maps, list(range(N_CORES)), _trace)
        LAST_EXEC_NS = res.exec_time_ns
        LAST_RESULTS = res
        rs = np.concatenate(
            [res.results[c]["rs"].T.reshape(-1) for c in range(N_CORES)]
        )
        return _finish_loss(rs, zi, zj)

    key = ("sym", NFULL, N_CORES, DIAG, WA, WB)
    if key not in _PROGRAM_CACHE:
        _PROGRAM_CACHE[key] = build_program_sym(NFULL, N_CORES, DIAG, WA, WB)
    nc = _PROGRAM_CACHE[key]

    n_mt = NFULL // P
    in_maps = []
    core_mtiles = []
    for c in range(N_CORES):
        mtiles = _mtiles_for_core(c, n_mt, N_CORES)
        core_mtiles.append(mtiles)
        blk = np.concatenate([zT[:, M * P : (M + 1) * P] for M in mtiles], axis=1)
        in_maps.append({"zT": zT, "blkT": np.ascontiguousarray(blk)})

    res = _run_with_retry(nc, in_maps, list(range(N_CORES)), _trace)
    LAST_EXEC_NS = res.exec_time_ns
    LAST_RESULTS = res

    rs_full = np.zeros(NFULL, dtype=np.float64)
    cs_tot = np.zeros((P, n_mt), dtype=np.float64)
    for c in range(N_CORES):
        rs_c = res.results[c]["rs"]  # [P, mt]
        for lm, M in enumerate(core_mtiles[c]):
            rs_full[M * P : (M + 1) * P] += rs_c[:, lm]
        cs_tot += res.results[c]["cs"]
    cs_tot[:, : DIAG // P] = 0.0
    rs_full += cs_tot.T.reshape(-1)
    return _finish_loss(rs_full, zi, zj)

